# revision 1
# baseline (speedup 1.0000x reference)
"""Trainium2 Bass kernel: conv3d(16,3x3x3,VALID) -> channel softmax -> 2x maxpool3d(2) twice.

Full inputs: x [8,3,96,96,96] f32, w [16,3,3,3,3] f32, b [16] f32.
Output: [8,16,23,23,23] f32.

Sharding: data-parallel over batch N=8 across 8 NeuronCores (1 sample/core).

Per-core algorithm (sample x_i [3,96,96,96] -> out_i [16,23,23,23]):
  Only conv outputs d,h,w in [0,92) survive the two maxpools (23*4=92), so we
  compute conv on a 92^3 grid, grouped as 23 d-quads x 23 h-quads x 92 w.

  Layout trick: one PSUM tile [128, 368] holds 8 h-quads x 16 channels on the
  partition axis (partition p = 16*g + c, g = h-quad index within a chunk of 8)
  and (h_local 4, w 92) on the free axis, for one conv-output depth d.
  The conv is a matmul with a block-diagonal stationary operand:
    lhsT [108, 64] = diag blocks of w[(cin,kd,kh), cout] per kw tap (K=27*4),
  accumulating 3 kw taps into PSUM via column-shifted views of one rhs tile.
  Two concurrent matmuls (tile_position col groups (0,0) and (0,64)) fill all
  128 partitions.

  Softmax+pool in log domain: y = (x+b) - ln(sum_c exp(x_c+b)) and
  maxpool(softmax) = exp(maxpool(y)) since exp is monotone. So:
    exp:  ACT e = exp(logits + b) -> SBUF as float32r (12-bit mantissa)
    sum:  PE  S = lambda * sum_c e, group-BROADCAST to all 128 partitions
          via a [128,128] block-ones lhsT; lambda = 2^-38 keeps ln's input
          inside the ACT Ln LUT's valid range [e^-44.5, e^44.5]
    ln:   ACT full-f32 ln(S) (f32r would round |ln S|~40 too coarsely)
    sub:  DVE y = logits - ln(S)
    pool: DVE reduce_max over w, then h_local, then d (all free-axis APs)
    out:  ACT exp(y_pool + b - 38*ln2), one 3-dim DMA per (dq, chunk).

  Conv matmuls are fp16 hi/lo 3-term (xh*wh + xh*wl + xl*wh, ~2e-5 rel):
  fp32 matmul is 4 cycles/row and float32r matmuls may only write PSUM
  partition 0 (ISA s3d3 check), which would kill the col-group concurrency.
  The sum matmul stays float32r at base partition 0 (1 cycle/row, N>=256).

  DMA: each dma_start costs ~0.76us fixed regardless of size, and DMA APs
  are limited to 3 dims with a contiguous last dim. So x is first staged
  into B[27 taps, d, h, w] in DRAM (54 big HBM->HBM copies, both HWDGE
  rings), after which each rhs im2col tile quarter loads as a single
  27-partition DMA (288 loads total instead of 2592 3-partition pieces).
"""

import numpy as np
from contextlib import ExitStack

import concourse.bass as bass
import concourse.bacc as bacc
import concourse.tile as tile
from concourse import mybir
from concourse.bass_utils import run_bass_kernel_spmd

F32 = mybir.dt.float32
F32R = mybir.dt.float32r
BF16 = mybir.dt.bfloat16
F16 = mybir.dt.float16

N_CORES = 8
CIN, S = 3, 96
COUT = 16
Q = 23          # pooled output size per dim
DU = 92         # conv positions used per dim (23*4)
NW = 94         # w extent loaded (92 + 2 halo for kw shifts)

CONV_MODE = "hilo"   # "f32r" | "hilo" (fp16 hi/lo 3-term)

_cache: dict = {}


def _emit(nc, xs, wls, ws_, wbc_, bias_, bias2_, out_, mode, chunks=(0, 8, 15),
          dq0s=None, ndq_cap=4, repeat=1, stage="full"):
    """Build the Tile kernel. xs: list of x dram APs (1 for f32r, 2 for bf16x2
    [hi, lo]); wls: list of lhsT dram APs ([108,3,64] each)."""
    S2 = S * S          # 9216
    S3 = S * S * S      # 884736
    if dq0s is None:
        dq0s = range(0, Q, 4)

    with tile.TileContext(nc) as tc, ExitStack() as ctx:
        consts = ctx.enter_context(tc.tile_pool(name="consts", bufs=1))
        rhsp = ctx.enter_context(tc.tile_pool(name="rhs", bufs=3))
        ep = ctx.enter_context(tc.tile_pool(name="e", bufs=3))
        ellp = ctx.enter_context(tc.tile_pool(name="ell", bufs=3))
        yp = ctx.enter_context(tc.tile_pool(name="y", bufs=3))
        wpp = ctx.enter_context(tc.tile_pool(name="wp", bufs=2))
        hpp = ctx.enter_context(tc.tile_pool(name="hp", bufs=2))
        finp = ctx.enter_context(tc.tile_pool(name="fin", bufs=2))
        outp = ctx.enter_context(tc.tile_pool(name="outt", bufs=2))
        psl = ctx.enter_context(tc.tile_pool(name="psl", bufs=3, space="PSUM"))
        pss = ctx.enter_context(tc.tile_pool(name="pss", bufs=1, space="PSUM"))

        rhs_dt = F32R if mode == "f32r" else F16
        
        # constants
        wlts = []
        for i, wl in enumerate(wls):
            t = consts.tile([108, 3, 64], rhs_dt, tag=f"wl{i}")
            nc.sync.dma_start(out=t, in_=wl[:])
            wlts.append(t)
        wst = consts.tile([128, 128], F32R, tag="ws")
        nc.sync.dma_start(out=wst, in_=ws_[:])
        biast = consts.tile([128, 1], F32, tag="bias")
        nc.sync.dma_start(out=biast, in_=bias_[:])
        biast2 = consts.tile([128, 1], F32, tag="bias2")
        nc.sync.dma_start(out=biast2, in_=bias2_[:])

        nx = len(xs)  # matmul terms per tap (1 or 3 -> hi/lo operand pairs)
        # term -> (x operand index, lhsT operand index)
        if mode == "f32r":
            terms = [(0, 0)]
        else:
            terms = [(0, 0), (0, 1), (1, 0)]  # xh*wh + xh*wl + xl*wh

        # --- staging: B[v][slot=(ci,kd,kh), d, h, w] = x[ci, d+kd, h+kh, w] ---
        # One HBM->HBM copy per slot (27 per operand). After staging, each
        # rhs tile quarter loads with a single 27-partition DMA (the 3-dim
        # DMA AP limit makes direct strided loads need 3-partition pieces,
        # and each dma_start costs ~0.76us fixed).
        DH, HH, HW = 94, 93, 96
        dramp = ctx.enter_context(tc.tile_pool(name="dram", bufs=1, space="DRAM"))
        Bs = []
        for v in range(nx):
            bt = dramp.tile([27, DH, HH, HW], rhs_dt, tag=f"B{v}")
            for ci in range(CIN):
                for kd in range(3):
                    for kh in range(3):
                        slot = 9 * ci + 3 * kd + kh
                        src = bass.AP(
                            tensor=xs[v],
                            offset=ci * S3 + kd * S2 + kh * S,
                            ap=[[S2, DH], [1, HH * HW]],
                        )
                        eng = nc.scalar if (slot % 2) else nc.sync
                        eng.dma_start(
                            out=bt[slot].rearrange("d h w -> d (h w)"), in_=src)
            Bs.append(bt)

        for _rep in range(repeat):
          for hq0 in chunks:
            for dq0 in dq0s:
                ndq = min(ndq_cap, Q - dq0)
                E = 4 * ndq  # depths staged in this rhs tile group (16 or 12)
                # --- load rhs im2col tiles [108, E, 4*96] for 4 d-quads ---
                # rhs[v][a]: operand v (hi/lo), half a (h-quad groups 4a..4a+3)
                # partition r = 27*g4 + 9*ci + 3*kd + kh; free = (d, h_local*96+w)
                EL = 1 if stage == "dmat" else E
                rhs = [[None, None] for _ in range(nx)]
                for v in range(nx):
                    for a in (0, 1):
                        t = rhsp.tile([108, 16, 4 * S], rhs_dt, tag=f"rhs{v}{a}")
                        rhs[v][a] = t
                        for g4 in range(4):
                            hq = hq0 + 4 * a + g4
                            src = bass.AP(
                                tensor=Bs[v].tensor,
                                offset=(Bs[v].offset
                                        + (4 * dq0) * HH * HW + (4 * hq) * HW),
                                ap=[[DH * HH * HW, 27], [HH * HW, EL], [1, 4 * S]],
                            )
                            eng = nc.scalar if (g4 % 2) else nc.sync
                            eng.dma_start(
                                out=t[27 * g4:27 * g4 + 27, 0:EL, :], in_=src)

                for dq in range(dq0, dq0 + ndq):
                    dsi0 = 4 * (dq - dq0)
                    if stage in ("dmao", "dmat"):
                        continue
                    hp = hpp.tile([128, 4, Q], F32)
                    if stage == "dma":
                        nc.vector.memset(hp, 0.0)
                    for pr in ((0, 1) if stage not in ("dma", "dmao") else ()):
                        logits = psl.tile([128, 2, 512], F32)
                        first = {(a, dl): True for a in (0, 1) for dl in (0, 1)}
                        nmm = 3 * len(terms)
                        cnt = {(a, dl): 0 for a in (0, 1) for dl in (0, 1)}
                        for dl in (0, 1):
                            dsi = dsi0 + 2 * pr + dl
                            for kw in range(3):
                                for a in (0, 1):
                                    for (xi, wi) in terms:
                                        lhsT = wlts[wi][:, kw, :]
                                        r = rhs[xi][a][:, dsi, :].rearrange(
                                            "p (hl w) -> p hl w", hl=4,
                                        )[:, :, kw:kw + DU]
                                        cnt[(a, dl)] += 1
                                        nc.tensor.matmul(
                                            out=logits[64 * a:64 * a + 64, dl, 0:368],
                                            lhsT=lhsT,
                                            rhs=r,
                                            start=first[(a, dl)],
                                            stop=(cnt[(a, dl)] == nmm),
                                            skip_group_check=True,
                                        )
                                        first[(a, dl)] = False
                        if stage == "conv":
                            wp0 = wpp.tile([128, 2, 4, Q], F32)
                            nc.vector.reduce_max(
                                out=wp0,
                                in_=logits[:, :, 0:368].rearrange(
                                    "p d (hl wq wl) -> p d hl wq wl",
                                    hl=4, wq=Q),
                                axis=mybir.AxisListType.X,
                            )
                            nc.vector.reduce_max(
                                out=hp[:, 2 * pr:2 * pr + 2, :],
                                in_=wp0.rearrange("p d hl wq -> p d wq hl"),
                                axis=mybir.AxisListType.X,
                            )
                            continue
                        # exp(logits + b) for both d of the pair, PSUM -> SBUF
                        e = ep.tile([128, 2, 368], F32R)
                        nc.scalar.activation(
                            out=e, in_=logits[:, :, 0:368],
                            func=mybir.ActivationFunctionType.Exp,
                            bias=biast[:, 0:1],
                        )
                        # per-group channel sums, broadcast to all 128
                        # partitions in one matmul: lhsT[k, p] = (k//16==p//16)
                        s = pss.tile([128, 2, 512], F32)
                        for dl in (0, 1):
                            nc.tensor.matmul(
                                out=s[:, dl, 0:368],
                                lhsT=wst,
                                rhs=e[:, dl, :],
                                start=True, stop=True,
                            )
                        # ln(s) in full fp32 (f32r would round |ln s|~40 too
                        # coarsely), then y = logits - ln(s) on DVE
                        ell = ellp.tile([128, 2, 368], F32)
                        nc.scalar.activation(
                            out=ell, in_=s[:, :, 0:368],
                            func=mybir.ActivationFunctionType.Ln,
                        )
                        y = yp.tile([128, 2, 368], F32)
                        nc.vector.tensor_tensor(
                            out=y, in0=logits[:, :, 0:368], in1=ell,
                            op=mybir.AluOpType.subtract,
                        )
                        # w-pool: [128, 2, 4, 23, 4] -> [128, 2, 4, 23]
                        wp = wpp.tile([128, 2, 4, Q], F32)
                        nc.vector.reduce_max(
                            out=wp,
                            in_=y.rearrange(
                                "p d (hl wq wl) -> p d hl wq wl", hl=4, wq=Q),
                            axis=mybir.AxisListType.X,
                        )
                        # h-pool: reduce over h_local -> hp[:, 2*pr:2*pr+2, :]
                        nc.vector.reduce_max(
                            out=hp[:, 2 * pr:2 * pr + 2, :],
                            in_=wp.rearrange("p d hl wq -> p d wq hl"),
                            axis=mybir.AxisListType.X,
                        )
                    # d-pool over the quad
                    fin = finp.tile([128, Q], F32)
                    nc.vector.reduce_max(
                        out=fin,
                        in_=hp.rearrange("p d wq -> p wq d"),
                        axis=mybir.AxisListType.X,
                    )
                    # back to probability domain, + bias inside exp
                    ot = outp.tile([128, Q], F32)
                    nc.scalar.activation(
                        out=ot, in_=fin,
                        func=mybir.ActivationFunctionType.Exp,
                        bias=biast2[:, 0:1],
                    )
                    if stage == "dmao":
                        continue
                    # SBUF side stays a plain [128, Q] AP (partition-major
                    # order is already g-major); the DRAM side carries the
                    # (g, c, w) pattern. Split-partition SBUF APs mislower.
                    dma_out_eng = nc.sync
                    dma_out_eng.dma_start(
                        out=out_[:][:, dq, hq0:hq0 + 8, :].rearrange(
                            "c g w -> g c w"),
                        in_=ot,
                    )


def _build(mode, chunks=(0, 8, 15), dq0s=None, ndq_cap=4, repeat=1, stage="full"):
    nc = bacc.Bacc(name="conv_softmax_pool")
    if mode == "f32r":
        xs = [nc.declare_dram_parameter("x", [CIN, S, S, S], F32R, isOutput=False)]
        wls = [nc.declare_dram_parameter("wl0", [108, 3, 64], F32R, isOutput=False)]
    else:
        xs = [
            nc.declare_dram_parameter("xh", [CIN, S, S, S], F16, isOutput=False),
            nc.declare_dram_parameter("xl", [CIN, S, S, S], F16, isOutput=False),
        ]
        wls = [
            nc.declare_dram_parameter("wl0", [108, 3, 64], F16, isOutput=False),
            nc.declare_dram_parameter("wl1", [108, 3, 64], F16, isOutput=False),
        ]
    ws_ = nc.declare_dram_parameter("ws", [128, 128], F32R, isOutput=False)
    wbc_ = None
    bias_ = nc.declare_dram_parameter("bias", [128, 1], F32, isOutput=False)
    bias2_ = nc.declare_dram_parameter("bias2", [128, 1], F32, isOutput=False)
    out_ = nc.declare_dram_parameter("out", [COUT, Q, Q, Q], F32, isOutput=True)
    _emit(nc, xs, wls, ws_, wbc_, bias_, bias2_, out_, mode, chunks=chunks,
          dq0s=dq0s, ndq_cap=ndq_cap, repeat=repeat, stage=stage)
    nc.finalize()
    return nc


def _host_prep(w, b, mode):
    """Build lhsT block-diagonal matrices and softmax helper matrices."""
    # wl[r, kw, m]: r = 27g + 9ci + 3kd + kh, m = 16g + c  (g = 0..3)
    def blockdiag(wm):  # wm [cout, cin, kd, kh, kw] float
        wl = np.zeros((108, 3, 64), np.float32)
        for g in range(4):
            for ci in range(CIN):
                for kd in range(3):
                    for kh in range(3):
                        wl[27 * g + 9 * ci + 3 * kd + kh, :, 16 * g:16 * g + 16] = \
                            wm[:, ci, kd, kh, :].T
        return wl

    # 2^-38 scale keeps ln(lambda*s) inside the ACT Ln LUT's valid input
    # range [e^-44.5, e^44.5]; compensated in the final exp bias.
    ws_ = np.zeros((128, 128), np.float32)
    for g in range(8):
        ws_[16 * g:16 * g + 16, 16 * g:16 * g + 16] = 2.0 ** -38
    wbc_ = None
    bias_ = np.tile(b.astype(np.float32), 8).reshape(128, 1)
    # y = logit - ln(lambda*s) = logit - ln s + 38ln2, so the final
    # exp needs bias2 = b - 38ln2 to recover exp(logit + b - ln s).
    bias2_ = bias_ - np.float32(38.0 * np.log(2.0))

    if mode == "f32r":
        wls = [blockdiag(w.astype(np.float32))]
    else:
        wh = w.astype(np.float32).astype(np.float16)
        wlo = (w.astype(np.float32) - wh.astype(np.float32)).astype(np.float16)
        wls = [blockdiag(wh.astype(np.float32)).astype(np.float16),
               blockdiag(wlo.astype(np.float32)).astype(np.float16)]
    return wls, ws_, wbc_, bias_, bias2_


def kernel(x, w, b):
    mode = CONV_MODE
    key = ("nc", mode)
    if key not in _cache:
        _cache[key] = _build(mode)
    nc = _cache[key]

    x = np.asarray(x, np.float32)
    w = np.asarray(w, np.float32)
    b = np.asarray(b, np.float32)
    wls, ws_, wbc_, bias_, bias2_ = _host_prep(w, b, mode)

    in_maps = []
    for i in range(N_CORES):
        m = {"ws": ws_, "bias": bias_, "bias2": bias2_}
        if mode == "f32r":
            m["x"] = np.ascontiguousarray(x[i])
            m["wl0"] = wls[0]
        else:
            xh = x[i].astype(np.float16)
            xl = (x[i] - xh.astype(np.float32)).astype(np.float16)
            m["xh"] = np.ascontiguousarray(xh)
            m["xl"] = np.ascontiguousarray(xl)
            m["wl0"] = wls[0]
            m["wl1"] = wls[1]
        in_maps.append(m)

    res = run_bass_kernel_spmd(nc, in_maps, core_ids=list(range(N_CORES)))
    return np.stack([r["out"] for r in res.results]).astype(np.float32)



# revision 8
# speedup vs baseline: 2.2584x; 2.2584x over previous
"""Trainium2 Bass kernel: conv3d(16,3x3x3,VALID) -> channel softmax -> 2x maxpool3d(2).

Full inputs: x [8,3,96,96,96] f32, w [16,3,3,3,3] f32, b [16] f32.
Output: [8,16,23,23,23] f32.

Sharding: data-parallel over batch N=8 across 8 NeuronCores (1 sample/core).

Per-core design (sample x_i [3,96,96,96] -> out_i [16,23,23,23]):

  Conv as 128-column banded matmuls in f32r (1 PE cycle/row at N>=256):
  columns m = (dl*32 + q*16 + c) pack 8 consecutive conv-d positions
  (dg = 4q + dl) x 16 cout; rows p = (ci, kd' in 0..9, kh) = 90 taps with
  kd' = dg + kd (d-banding shares rows across the 8 d columns), and kw
  realized as 3 column-shifted views of one rhs tile accumulated in PSUM.
  Out free = (4 h-rows, 92 w) = 368 within one PSUM bank. 12 d-blocks
  (d0 = 0..80 step 8, then 84) x 23 h-quads x 3 kw matmuls.

  The rhs tile [90, 94*96] loads DIRECTLY from x (no staging, no im2col
  duplication): per (block, ci) one DMA with in-AP [[9216,10],[96,3],
  [1,9024]] - partitions (kd',kh), one contiguous (h,w) span per
  partition; the kh shift is absorbed into each partition's base offset
  so every partition shares one free view per (hq, kw).

  Softmax in log domain, pools before the final exp (exp is monotone):
    exp:  ACT e = exp(logits + b - 35ln2) -> SBUF bf16 (the 2^-35
          scale keeps ln input under the ACT Ln range limit 2^64)
    sum:  PE  s[32sl+j] = sum_c e for group j<8 (cols 8..31 sum all
          partitions - a junk-guard so ln stays finite), 32-aligned col
          strips, 4 hq slots per PSUM bank
    ln:   ACT ell = ln(s) on the standing tile -> SBUF f32r, 1 per 4 hq
    sub:  PE  logits -= ell[32sl+dg(m)] via accumulating matmul with a
          -1-selector lhsT [128,128] f32r (start=False onto the conv bank)
    pool: DVE reduce_max over w then h, f32 throughout (y is offset by
          35ln2, too large for comfortable f16 ulps)
    dmax: d-quad max needs partition folds: 3 SBUF->SBUF re-base DMAs +
          3 same-base tensor_tensor maxes (cross-base SBUF pairs and
          GPSIMD/PSUM are rejected by the BIR verifier)
    out:  ACT exp(v2 + b - 35ln2) [32,529] f32, one 3-dim DMA per block.

  PSUM: conv tiles [128,2,512] bufs=3 + s-standing [128,512] bufs=2 =
  8 banks exactly.
"""

import numpy as np
from contextlib import ExitStack

import concourse.bass as bass
import concourse.bacc as bacc
import concourse.tile as tile
from concourse import mybir
from concourse.bass_utils import run_bass_kernel_spmd

F32 = mybir.dt.float32
F32R = mybir.dt.float32r
BF16 = mybir.dt.bfloat16
F16 = mybir.dt.float16

N_CORES = 8
CIN, S = 3, 96
COUT = 16
Q = 23
S2 = S * S
S3 = S * S * S
D0S = [8 * b for b in range(11)] + [84]
LN_LAM = -35.0 * float(np.log(2.0))

_cache: dict = {}


def _dg_of_m(m):
    return 4 * ((m % 32) // 16) + m // 32


def _emit(nc, x_, wl_, ones32_, negsel_, biase_, bias2_, out_):
    AF = mybir.ActivationFunctionType
    ALU = mybir.AluOpType
    AX = mybir.AxisListType

    with tile.TileContext(nc) as tc, ExitStack() as ctx:
        consts = ctx.enter_context(tc.tile_pool(name="consts", bufs=1))
        rhsp = ctx.enter_context(tc.tile_pool(name="rhs", bufs=2))
        ep = ctx.enter_context(tc.tile_pool(name="e", bufs=3))
        ellp = ctx.enter_context(tc.tile_pool(name="ell", bufs=2))
        wpp = ctx.enter_context(tc.tile_pool(name="wp", bufs=3))
        hpp = ctx.enter_context(tc.tile_pool(name="hp", bufs=2))
        dpp = ctx.enter_context(tc.tile_pool(name="dp", bufs=2))
        finp = ctx.enter_context(tc.tile_pool(name="fin", bufs=2))
        psP = ctx.enter_context(tc.tile_pool(name="psP", bufs=3, space="PSUM"))
        psS = ctx.enter_context(tc.tile_pool(name="psS", bufs=2, space="PSUM"))

        # Preload the one ACT function table that serves Exp AND Ln
        # (natural_log_exp_and_others, set id 6) so the table-load pass
        # doesn't bounce between per-function tables on every Exp<->Ln
        # alternation (1283ns per reload).
        nc.scalar.add_instruction(mybir.InstLoadActFuncSet(
            name=nc.get_next_instruction_name(),
            act_func_set_id=6, ins=[], outs=[]))

        wlt = consts.tile([90, 3, 128], F32R, tag="wl")
        nc.sync.dma_start(out=wlt, in_=wl_[:])
        ones32t = consts.tile([128, 32], F16, tag="ones32")
        nc.sync.dma_start(out=ones32t, in_=ones32_[:])
        negselt = consts.tile([128, 4, 128], F32R, tag="negsel")
        nc.sync.dma_start(out=negselt, in_=negsel_[:])
        biaset = consts.tile([128, 1], F32, tag="biase")
        nc.sync.dma_start(out=biaset, in_=biase_[:])
        bias2t = consts.tile([32, 1], F32, tag="bias2")
        nc.sync.dma_start(out=bias2t, in_=bias2_[:])

        for d0 in D0S:
            rhst = rhsp.tile([90, 94 * S], F32R, tag="rhs")
            for ci in range(CIN):
                src = bass.AP(
                    tensor=x_,
                    offset=ci * S3 + d0 * S2,
                    ap=[[S2, 10], [S, 3], [1, 94 * S]],
                )
                nc.sync.dma_start(out=rhst[30 * ci:30 * ci + 30, :], in_=src)
            rh = rhst.rearrange("p (h w) -> p h w", w=S)

            HP = hpp.tile([128, Q, Q], F32, tag="HP")
            for sp in range(6):
                hqs = list(range(4 * sp, min(4 * sp + 4, Q)))
                npart = 32 * len(hqs)
                s_std = psS.tile([128, 512], F32, tag="s")
                Ps = []
                for rr in range(0, len(hqs), 2):
                    rhqs = hqs[rr:rr + 2]
                    nh = len(rhqs)
                    P = psP.tile([128, 2, 512], F32, tag="P")
                    Ps.append((P, rhqs))
                    for kw in range(3):
                        lhsT = wlt[:, kw, :]
                        for j, hq in enumerate(rhqs):
                            nc.tensor.matmul(
                                out=P[:, j, 0:368],
                                lhsT=lhsT,
                                rhs=rh[:, 4 * hq:4 * hq + 4, kw:kw + 92],
                                start=(kw == 0),
                                stop=(kw == 2),
                                skip_group_check=True,
                            )
                    e = ep.tile([128, 2, 368], BF16, tag="e")
                    nc.scalar.activation(
                        out=e[:, 0:nh, :], in_=P[:, 0:nh, 0:368],
                        func=AF.Exp, bias=biaset[:, 0:1],
                    )
                    for j, hq in enumerate(rhqs):
                        sl = hq % 4
                        nc.tensor.matmul(
                            out=s_std[32 * sl:32 * sl + 32, 0:368],
                            lhsT=ones32t,
                            rhs=e[:, j, :],
                            start=True, stop=True,
                            skip_group_check=True,
                            tile_position=(0, 32 * sl),
                        )
                ell = ellp.tile([128, 368], F32R, tag="ell")
                with nc.allow_low_precision(reason="log-magnitudes; 2e-2 gate"):
                    nc.scalar.activation(
                        out=ell[0:npart, :], in_=s_std[0:npart, 0:368],
                        func=AF.Ln,
                    )
                for P, rhqs in Ps:
                    nh = len(rhqs)
                    for j, hq in enumerate(rhqs):
                        sl = hq % 4
                        nc.tensor.matmul(
                            out=P[:, j, 0:368],
                            lhsT=negselt[0:npart, sl, :],
                            rhs=ell[0:npart, :],
                            start=False, stop=True,
                            skip_group_check=True,
                        )
                    wp = wpp.tile([128, 2, 4, Q], F32, tag="wp")
                    nc.vector.reduce_max(
                        out=wp[:, 0:nh],
                        in_=P[:, 0:nh, 0:368].rearrange(
                            "p r (hl wq wl) -> p r hl wq wl", hl=4, wq=Q),
                        axis=AX.X,
                    )
                    nc.vector.reduce_max(
                        out=HP[:, rhqs[0]:rhqs[0] + nh, :],
                        in_=wp[:, 0:nh].rearrange("p r hl wq -> p r wq hl"),
                        axis=AX.X,
                    )
            # d-quad max across partition strips (m = dl*32 + q*16 + c)
            HPf = HP.rearrange("p a b -> p (a b)")
            t1 = dpp.tile([32, Q * Q], F32, tag="t1")
            t2 = dpp.tile([32, Q * Q], F32, tag="t2")
            t3 = dpp.tile([32, Q * Q], F32, tag="t3")
            nc.sync.dma_start(out=t1, in_=HPf[32:64, :])
            nc.sync.dma_start(out=t2, in_=HPf[64:96, :])
            nc.sync.dma_start(out=t3, in_=HPf[96:128, :])
            va = dpp.tile([32, Q * Q], F32, tag="va")
            vb = dpp.tile([32, Q * Q], F32, tag="vb")
            nc.vector.tensor_tensor(out=va, in0=HPf[0:32, :], in1=t1, op=ALU.max)
            nc.vector.tensor_tensor(out=vb, in0=t2, in1=t3, op=ALU.max)
            v2 = dpp.tile([32, Q * Q], F32, tag="v2")
            nc.vector.tensor_tensor(out=v2, in0=va, in1=vb, op=ALU.max)
            fe = finp.tile([32, Q * Q], F32, tag="fe")
            nc.scalar.activation(out=fe, in_=v2, func=AF.Exp,
                                 bias=bias2t[:, 0:1])
            q0 = d0 // 4
            dst = bass.AP(tensor=out_, offset=q0 * Q * Q,
                          ap=[[Q * Q, 2], [Q * Q * Q, COUT], [1, Q * Q]])
            nc.sync.dma_start(out=dst, in_=fe)


def _build():
    nc = bacc.Bacc(name="conv_softmax_pool")
    x_ = nc.declare_dram_parameter("x", [CIN, S, S, S], F32R, isOutput=False)
    wl_ = nc.declare_dram_parameter("wl", [90, 3, 128], F32R, isOutput=False)
    ones32_ = nc.declare_dram_parameter("ones32", [128, 32], F16, isOutput=False)
    negsel_ = nc.declare_dram_parameter("negsel", [128, 4, 128], F32R,
                                        isOutput=False)
    biase_ = nc.declare_dram_parameter("biase", [128, 1], F32, isOutput=False)
    bias2_ = nc.declare_dram_parameter("bias2", [32, 1], F32, isOutput=False)
    out_ = nc.declare_dram_parameter("out", [COUT, Q, Q, Q], F32, isOutput=True)
    _emit(nc, x_, wl_, ones32_, negsel_, biase_, bias2_, out_)
    nc.finalize()
    return nc


def _host_prep(w, b):
    wl = np.zeros((90, 3, 128), np.float32)
    for ci in range(CIN):
        for kdp in range(10):
            for kh in range(3):
                p = ci * 30 + kdp * 3 + kh
                for m in range(128):
                    dg = _dg_of_m(m)
                    kd = kdp - dg
                    if 0 <= kd <= 2:
                        wl[p, :, m] = w[m % 16, ci, kd, kh, :]
    ones32 = np.zeros((128, 32), np.float16)
    for k in range(128):
        dgk = _dg_of_m(k)
        for j in range(32):
            ones32[k, j] = 1.0 if (j >= 8 or dgk == j) else 0.0
    negsel = np.zeros((128, 4, 128), np.float32)
    for sl in range(4):
        for m in range(128):
            negsel[32 * sl + _dg_of_m(m), sl, m] = -1.0
    biase = np.array([b[m % 16] + LN_LAM for m in range(128)],
                     np.float32).reshape(128, 1)
    bias2 = np.array([b[m % 16] + LN_LAM for m in range(32)],
                     np.float32).reshape(32, 1)
    return wl, ones32, negsel, biase, bias2


def kernel(x, w, b):
    if "nc" not in _cache:
        _cache["nc"] = _build()
    nc = _cache["nc"]

    x = np.asarray(x, np.float32)
    w = np.asarray(w, np.float32)
    b = np.asarray(b, np.float32)
    wl, ones32, negsel, biase, bias2 = _host_prep(w, b)

    in_maps = []
    for i in range(N_CORES):
        in_maps.append({
            "x": np.ascontiguousarray(x[i]),
            "wl": wl, "ones32": ones32, "negsel": negsel,
            "biase": biase, "bias2": bias2,
        })

    res = run_bass_kernel_spmd(nc, in_maps, core_ids=list(range(N_CORES)))
    return np.stack([r["out"] for r in res.results]).astype(np.float32)


# revision 21
# speedup vs baseline: 3.2191x; 1.4254x over previous
"""Trainium2 Bass kernel: conv3d(16,3x3x3,VALID) -> channel softmax -> 2x maxpool3d(2).

Full inputs: x [8,3,96,96,96] f32, w [16,3,3,3,3] f32, b [16] f32.
Output: [8,16,23,23,23] f32.

Sharding: data-parallel over batch N=8 across 8 NeuronCores (1 sample/core).

Per-core design (sample x_i [3,96,96,96] -> out_i [16,23,23,23]):

  Conv as 128-column banded matmuls in f32r (1 PE cycle/row at N>=256):
  columns m = (dl*32 + q*16 + c) pack 8 consecutive conv-d positions
  (dg = 4q + dl) x 16 cout; rows p = (ci, kd' in 0..9, kh) = 90 taps with
  kd' = dg + kd (d-banding shares rows across the 8 d columns), and kw
  realized as 3 column-shifted views of one rhs tile accumulated in PSUM.
  Out free = (4 h-rows, 92 w) = 368 within one PSUM bank. 12 d-blocks
  (d0 = 0..80 step 8, then 84) x 23 h-quads x 3 kw matmuls.

  The rhs tile [90, 94*96] loads DIRECTLY from x (no staging, no im2col
  duplication): per (block, ci) one DMA with in-AP [[9216,10],[96,3],
  [1,9024]] - partitions (kd',kh), one contiguous (h,w) span per
  partition; the kh shift is absorbed into each partition's base offset
  so every partition shares one free view per (hq, kw).

  Softmax in log domain, pools before the final exp (exp is monotone):
    exp:  ACT e = exp(logits + b - 35ln2) -> SBUF bf16 (the 2^-35
          scale keeps ln input under the ACT Ln range limit 2^64)
    sum:  PE  s[32j+g] = sum_c e for group g<8 (cols 8..31 sum all
          partitions - a junk-guard so ln stays finite), 32-aligned col
          strips, the round's 2 hq slots in one PSUM bank
    ln:   ACT ell = ln(s[0:64]) -> SBUF f32r, one per round (keeping the
          normalize chain inside a round maximizes PSUM pipelining)
    sub:  PE  logits -= ell[32sl+dg(m)] via accumulating matmul with a
          -1-selector lhsT [128,128] f32r (start=False onto the conv bank)
    pool: DVE reduce_max over w then h, f32 throughout (y is offset by
          35ln2, too large for comfortable f16 ulps)
    dmax: d-quad max needs partition folds: 3 SBUF->SBUF re-base DMAs +
          3 same-base tensor_tensor maxes (cross-base SBUF pairs and
          GPSIMD/PSUM are rejected by the BIR verifier)
    out:  ACT exp(v2 + b - 35ln2) [32,529] f32, one 3-dim DMA per block.

  PSUM: conv tiles [128,2,512] bufs=3 + s [64,512] bufs=2 = 8 banks.
"""

import numpy as np
from contextlib import ExitStack

import concourse.bass as bass
import concourse.bacc as bacc
import concourse.tile as tile
from concourse import mybir
from concourse.bass_utils import run_bass_kernel_spmd

F32 = mybir.dt.float32
F32R = mybir.dt.float32r
BF16 = mybir.dt.bfloat16
F16 = mybir.dt.float16

N_CORES = 8
CIN, S = 3, 96
COUT = 16
Q = 23
S2 = S * S
S3 = S * S * S
D0S = [8 * b for b in range(11)] + [84]
LN_LAM = -35.0 * float(np.log(2.0))

_cache: dict = {}


def _dg_of_m(m):
    return 4 * ((m % 32) // 16) + m // 32


def _emit(nc, x_, wl_, ones32_, negsel_, biase_, bias2_, out_):
    AF = mybir.ActivationFunctionType
    ALU = mybir.AluOpType
    AX = mybir.AxisListType

    with tile.TileContext(nc) as tc, ExitStack() as ctx:
        consts = ctx.enter_context(tc.tile_pool(name="consts", bufs=1))
        ep = ctx.enter_context(tc.tile_pool(name="e", bufs=3))
        ellp = ctx.enter_context(tc.tile_pool(name="ell", bufs=2))
        wpp = ctx.enter_context(tc.tile_pool(name="wp", bufs=3))
        hph_p = ctx.enter_context(tc.tile_pool(name="hph", bufs=3))
        hpp = ctx.enter_context(tc.tile_pool(name="hp", bufs=2))
        dpp = ctx.enter_context(tc.tile_pool(name="dp", bufs=3))
        finp = ctx.enter_context(tc.tile_pool(name="fin", bufs=2))
        psP = ctx.enter_context(tc.tile_pool(name="psP", bufs=3, space="PSUM"))
        psS = ctx.enter_context(tc.tile_pool(name="psS", bufs=2, space="PSUM"))

        # Preload the one ACT function table that serves Exp AND Ln
        # (natural_log_exp_and_others, set id 6) so the table-load pass
        # doesn't bounce between per-function tables on every Exp<->Ln
        # alternation (1283ns per reload).
        nc.scalar.add_instruction(mybir.InstLoadActFuncSet(
            name=nc.get_next_instruction_name(),
            act_func_set_id=6, ins=[], outs=[]))

        wlt = consts.tile([90, 3, 128], F32R, tag="wl")
        nc.gpsimd.dma_start(out=wlt, in_=wl_[:])
        ones32t = consts.tile([128, 32], F16, tag="ones32")
        nc.gpsimd.dma_start(out=ones32t, in_=ones32_[:])
        negselt = consts.tile([64, 2, 128], F32R, tag="negsel")
        nc.gpsimd.dma_start(out=negselt, in_=negsel_[:])
        biaset = consts.tile([128, 1], F32, tag="biase")
        nc.gpsimd.dma_start(out=biaset, in_=biase_[:])
        bias2t = consts.tile([32, 1], F32, tag="bias2")
        nc.gpsimd.dma_start(out=bias2t, in_=bias2_[:])

        HPs = []
        for v in range(12):
            HPv = consts.tile([128, Q, Q], F32, tag=f"HP{v}", name=f"HP{v}")
            HPs.append(HPv)
        rhst0 = consts.tile([90, 94 * S], F32R, tag="rhs0")
        rhst1 = consts.tile([90, 94 * S], F32R, tag="rhs1")
        rhst2 = consts.tile([90, 94 * S], F32R, tag="rhs2")
        rhsts = [rhst0, rhst1, rhst2]
        for bi, d0 in enumerate(D0S):
            rhst = rhsts[bi % 3]
            with tc.high_priority():
                for ci in range(CIN):
                    src = bass.AP(
                        tensor=x_,
                        offset=ci * S3 + d0 * S2,
                        ap=[[S2, 10], [S, 3], [1, 94 * S]],
                    )
                    nc.sync.dma_start(out=rhst[30 * ci:30 * ci + 30, :], in_=src)
            rh = rhst.rearrange("p (h w) -> p h w", w=S)

            HP = HPs[bi]
            for r0 in range(0, Q, 2):
                rhqs = [r0] + ([r0 + 1] if r0 + 1 < Q else [])
                nh = len(rhqs)
                npart = 32 * nh
                P = psP.tile([128, 2, 512], F32, tag="P")
                s_std = psS.tile([64, 512], F32, tag="s")
                for kw in range(3):
                    lhsT = wlt[:, kw, :]
                    for j, hq in enumerate(rhqs):
                        nc.tensor.matmul(
                            out=P[:, j, 0:368],
                            lhsT=lhsT,
                            rhs=rh[:, 4 * hq:4 * hq + 4, kw:kw + 92],
                            start=(kw == 0),
                            stop=(kw == 2),
                            skip_group_check=True,
                        )
                e = ep.tile([128, 2, 368], BF16, tag="e")
                nc.scalar.activation(
                    out=e[:, 0:nh, :], in_=P[:, 0:nh, 0:368],
                    func=AF.Exp, bias=biaset[:, 0:1],
                )
                for j, hq in enumerate(rhqs):
                    nc.tensor.matmul(
                        out=s_std[32 * j:32 * j + 32, 0:368],
                        lhsT=ones32t,
                        rhs=e[:, j, :],
                        start=True, stop=True,
                        skip_group_check=True,
                        tile_position=(0, 32 * j),
                    )
                ell = ellp.tile([64, 368], F32R, tag="ell")
                with nc.allow_low_precision(reason="log-magnitudes; 2e-2 gate"):
                    nc.scalar.activation(
                        out=ell[0:npart, :], in_=s_std[0:npart, 0:368],
                        func=AF.Ln,
                    )
                for j, hq in enumerate(rhqs):
                    nc.tensor.matmul(
                        out=P[:, j, 0:368],
                        lhsT=negselt[0:npart, j, :],
                        rhs=ell[0:npart, :],
                        start=False, stop=True,
                        skip_group_check=True,
                    )
                wp = wpp.tile([128, 2, 4, Q], F32, tag="wp")
                nc.vector.reduce_max(
                    out=wp[:, 0:nh],
                    in_=P[:, 0:nh, 0:368].rearrange(
                        "p r (hl wq wl) -> p r hl wq wl", hl=4, wq=Q),
                    axis=AX.X,
                )
                nc.vector.reduce_max(
                    out=HP[:, r0:r0 + nh, :],
                    in_=wp[:, 0:nh].rearrange("p r hl wq -> p r wq hl"),
                    axis=AX.X,
                )

        # Deferred block tails, pipelined at kernel end: d-quad max across
        # partition strips (m = dl*32 + q*16 + c), final exp, output DMA.
        # Cross-base SBUF pairs are illegal, so re-base strips via DMA.
        for bi, d0 in enumerate(D0S):
            HPf = HPs[bi].rearrange("p a b -> p (a b)")
            th = dpp.tile([64, Q * Q], F32, tag="th")
            eng = nc.sync if bi % 2 == 0 else nc.gpsimd
            eng.dma_start(out=th, in_=HPf[64:128, :])
            u = dpp.tile([64, Q * Q], F32, tag="u")
            nc.vector.tensor_tensor(out=u, in0=HPf[0:64, :], in1=th,
                                    op=ALU.max)
            t2 = dpp.tile([32, Q * Q], F32, tag="t2")
            eng.dma_start(out=t2, in_=u[32:64, :])
            v2 = dpp.tile([32, Q * Q], F32, tag="v2")
            nc.vector.tensor_tensor(out=v2, in0=u[0:32, :], in1=t2,
                                    op=ALU.max)
            fe = finp.tile([32, Q * Q], F32, tag="fe")
            nc.scalar.activation(out=fe, in_=v2, func=AF.Exp,
                                 bias=bias2t[:, 0:1])
            q0 = d0 // 4
            dst = bass.AP(tensor=out_, offset=q0 * Q * Q,
                          ap=[[Q * Q, 2], [Q * Q * Q, COUT], [1, Q * Q]])
            eng.dma_start(out=dst, in_=fe)



def _build():
    nc = bacc.Bacc(name="conv_softmax_pool")
    x_ = nc.declare_dram_parameter("x", [CIN, S, S, S], F32R, isOutput=False)
    wl_ = nc.declare_dram_parameter("wl", [90, 3, 128], F32R, isOutput=False)
    ones32_ = nc.declare_dram_parameter("ones32", [128, 32], F16, isOutput=False)
    negsel_ = nc.declare_dram_parameter("negsel", [64, 2, 128], F32R,
                                        isOutput=False)
    biase_ = nc.declare_dram_parameter("biase", [128, 1], F32, isOutput=False)
    bias2_ = nc.declare_dram_parameter("bias2", [32, 1], F32, isOutput=False)
    out_ = nc.declare_dram_parameter("out", [COUT, Q, Q, Q], F32, isOutput=True)
    _emit(nc, x_, wl_, ones32_, negsel_, biase_, bias2_, out_)
    nc.finalize()
    return nc


def _host_prep(w, b):
    wl = np.zeros((90, 3, 128), np.float32)
    for ci in range(CIN):
        for kdp in range(10):
            for kh in range(3):
                p = ci * 30 + kdp * 3 + kh
                for m in range(128):
                    dg = _dg_of_m(m)
                    kd = kdp - dg
                    if 0 <= kd <= 2:
                        wl[p, :, m] = w[m % 16, ci, kd, kh, :]
    ones32 = np.zeros((128, 32), np.float16)
    for k in range(128):
        dgk = _dg_of_m(k)
        for j in range(32):
            ones32[k, j] = 1.0 if (j >= 8 or dgk == j) else 0.0
    negsel = np.zeros((64, 2, 128), np.float32)
    for sl in range(2):
        for m in range(128):
            negsel[32 * sl + _dg_of_m(m), sl, m] = -1.0
    biase = np.array([b[m % 16] + LN_LAM for m in range(128)],
                     np.float32).reshape(128, 1)
    bias2 = np.array([b[m % 16] + LN_LAM for m in range(32)],
                     np.float32).reshape(32, 1)
    return wl, ones32, negsel, biase, bias2


def kernel(x, w, b):
    if "nc" not in _cache:
        _cache["nc"] = _build()
    nc = _cache["nc"]

    x = np.asarray(x, np.float32)
    w = np.asarray(w, np.float32)
    b = np.asarray(b, np.float32)
    wl, ones32, negsel, biase, bias2 = _host_prep(w, b)

    in_maps = []
    for i in range(N_CORES):
        in_maps.append({
            "x": np.ascontiguousarray(x[i]),
            "wl": wl, "ones32": ones32, "negsel": negsel,
            "biase": biase, "bias2": bias2,
        })

    res = run_bass_kernel_spmd(nc, in_maps, core_ids=list(range(N_CORES)))
    return np.stack([r["out"] for r in res.results]).astype(np.float32)
